# revision 7
# baseline (speedup 1.0000x reference)
"""LocalGNNCliqueLine Trainium2 kernel: 8-way node-sharded chained GSO matmuls.

Reference (B=4, N1=8192, N2=4096, DEG=4, F0=1, F1=F2=16, K=3, R0=16, R1=2):
  h1 = relu(gfilter(x, S1, W10, b10))          # [B,16,N1]
  h2 = relu(gfilter(h1, S1, W11, b11))         # [B,16,N1]
  q  = relu-max-pool(h2, incid_idx)            # [B,16,N2]
  h3 = relu(gfilter(q, S2, W20, b20))          # [B,16,N2]
  y  = (relu(h3^T @ R0W + R0b) @ R1W + R1b)^T  # [B,2,N2]

Sharding: S1/S2 column(=output node)-sharded across 8 cores. Every GSO tap is
computed as out^T[n2, m] = sum_n1 S[n1, n2] * z^T[n1, m] with the S tile as
the PE stationary operand, so outputs stay node-on-partition and chain into
the next tap with no transposes. AllGather between taps. Feature mixing,
pooling and the readout MLP run on slab rows only via PE transpose + small
block-diagonal matmuls. Host staging pre-transposes x, slices the slabs, and
builds the block-diagonal weight matrices.
"""

import numpy as np

import concourse.bass as bass
import concourse.mybir as mybir
from concourse.bass_utils import run_bass_kernel_spmd

F32 = mybir.dt.float32
I32 = mybir.dt.int32

NC = 8
CORE_IDS = list(range(NC))
B, N1, N2, DEG = 4, 8192, 4096, 4
F1 = 16
M4 = B                       # 4: stacked rows for layer-1 taps (F0=1)
M = B * F1                   # 64
SLAB1 = N1 // NC             # 1024
SLAB2 = N2 // NC             # 512
C1 = N1 // 128               # 64
C2 = N2 // 128               # 32
J1 = SLAB1 // 128            # 8
J2 = SLAB2 // 128            # 4
NBUF = 16

RG = [CORE_IDS]
_CACHE = {}
DEBUG = False


def _host_prep(x, S1, S2, incid_idx, W10, W11, W20, b10, b11, b20,
               R0W, R0b, R1W, R1b):
    f = np.float32
    xT = np.ascontiguousarray(np.asarray(x, f)[:, 0, :].T)        # [N1, 4]

    w10cat = np.zeros((12, 64), f)                                 # rows (k,b)
    for k in range(3):
        for b in range(4):
            w10cat[k * 4 + b, b * 16:(b + 1) * 16] = np.asarray(W10, f)[k, 0, :]

    def blockdiag3(W):   # [3,16,16] -> [64, 192]; rows (b,f), k-major cols (b,g)
        out = np.zeros((64, 192), f)
        for k in range(3):
            for b in range(4):
                out[b * 16:(b + 1) * 16,
                    k * 64 + b * 16:k * 64 + (b + 1) * 16] = np.asarray(W, f)[k]
        return out

    def blockdiag1(W):   # [16, r] -> [64, 4r]
        W = np.asarray(W, f)
        r = W.shape[1]
        out = np.zeros((64, 4 * r), f)
        for b in range(4):
            out[b * 16:(b + 1) * 16, b * r:(b + 1) * r] = W
        return out

    w11r = blockdiag3(W11)
    w20r = blockdiag3(W20)
    r0wr = blockdiag1(R0W)                                         # [64, 64]
    r1wr = blockdiag1(R1W)                                         # [64, 8]

    bvec = np.zeros((64, 8), f)
    bvec[:, 0] = np.tile(np.asarray(b10, f), 4)
    bvec[:, 1] = np.tile(np.asarray(b11, f), 4)
    bvec[:, 2] = np.tile(np.asarray(b20, f), 4)
    bvec[:, 3] = np.tile(np.asarray(R0b, f), 4)
    bvec[:8, 4] = np.tile(np.asarray(R1b, f), 4)

    ident = np.eye(128, dtype=f)
    S1 = np.asarray(S1, f)
    S2 = np.asarray(S2, f)
    incid = np.asarray(incid_idx, np.int32)

    in_maps = []
    for c in range(NC):
        in_maps.append({
            "s1s": np.ascontiguousarray(S1[:, c * SLAB1:(c + 1) * SLAB1]),
            "s2s": np.ascontiguousarray(S2[:, c * SLAB2:(c + 1) * SLAB2]),
            "xT": xT,
            "xTs": np.ascontiguousarray(xT[c * SLAB1:(c + 1) * SLAB1]),
            "incs": np.ascontiguousarray(incid[c * SLAB2:(c + 1) * SLAB2]),
            "w10cat": w10cat, "w11r": w11r, "w20r": w20r,
            "r0wr": r0wr, "r1wr": r1wr, "bvec": bvec, "ident": ident,
        })
    return in_maps


class Op:
    __slots__ = ("engine", "fn", "deps", "extra_waits", "inc", "is_async",
                 "stage", "sem", "val")

    def __init__(self, engine, fn, deps=(), extra_waits=(), inc=1,
                 is_async=False, stage=0):
        self.engine = engine
        self.fn = fn
        self.deps = [d for d in deps if d is not None]
        self.extra_waits = list(extra_waits)
        self.inc = inc
        self.is_async = is_async
        self.stage = stage
        self.sem = None
        self.val = None


def _build_nc():
    nc = bass.Bass(num_devices=NC)

    s1s = nc.declare_dram_parameter("s1s", [N1, SLAB1], F32, isOutput=False)
    s2s = nc.declare_dram_parameter("s2s", [N2, SLAB2], F32, isOutput=False)
    xT_d = nc.declare_dram_parameter("xT", [N1, M4], F32, isOutput=False)
    xTs_d = nc.declare_dram_parameter("xTs", [SLAB1, M4], F32, isOutput=False)
    incs_d = nc.declare_dram_parameter("incs", [SLAB2, DEG], I32, isOutput=False)
    w10cat_d = nc.declare_dram_parameter("w10cat", [12, 64], F32, isOutput=False)
    w11r_d = nc.declare_dram_parameter("w11r", [64, 192], F32, isOutput=False)
    w20r_d = nc.declare_dram_parameter("w20r", [64, 192], F32, isOutput=False)
    r0wr_d = nc.declare_dram_parameter("r0wr", [64, 64], F32, isOutput=False)
    r1wr_d = nc.declare_dram_parameter("r1wr", [64, 8], F32, isOutput=False)
    bvec_d = nc.declare_dram_parameter("bvec", [64, 8], F32, isOutput=False)
    ident_d = nc.declare_dram_parameter("ident", [128, 128], F32, isOutput=False)
    y_d = nc.declare_dram_parameter("y", [8, SLAB2], F32, isOutput=True)

    z1_in = nc.dram_tensor("z1_in", [SLAB1, M4], F32)
    z1_out = nc.dram_tensor("z1_out", [N1, M4], F32, addr_space="Shared")
    h_in = nc.dram_tensor("h_in", [SLAB1, M], F32)
    h_out = nc.dram_tensor("h_out", [N1, M], F32, addr_space="Shared")
    u1_in = nc.dram_tensor("u1_in", [SLAB1, M], F32)
    u1_out = nc.dram_tensor("u1_out", [N1, M], F32, addr_space="Shared")
    p_in = nc.dram_tensor("p_in", [SLAB1, M], F32)
    p_out = nc.dram_tensor("p_out", [N1, M], F32, addr_space="Shared")
    q_in = nc.dram_tensor("q_in", [SLAB2, M], F32)
    q_out = nc.dram_tensor("q_out", [N2, M], F32, addr_space="Shared")
    v1_in = nc.dram_tensor("v1_in", [SLAB2, M], F32)
    v1_out = nc.dram_tensor("v1_out", [N2, M], F32, addr_space="Shared")

    from contextlib import ExitStack
    ctx = ExitStack()
    sb = lambda name, shape, dt=F32: ctx.enter_context(  # noqa: E731
        nc.sbuf_tensor(name, shape, dt))
    psa = lambda name, shape: ctx.enter_context(  # noqa: E731
        nc.psum_tensor(name, shape, F32))
    mksem = lambda name: ctx.enter_context(nc.semaphore(name))  # noqa: E731

    with ctx:
        sring = sb("sring", [128, NBUF * SLAB1])
        xT_sb = sb("xT_sb", [128, C1 * M4])
        z1T_sb = sb("z1T_sb", [128, C1 * M4])
        xTs_sb = sb("xTs_sb", [128, J1 * M4])
        z1Ts_sb = sb("z1Ts_sb", [128, J1 * M4])
        z2Ts_sb = sb("z2Ts_sb", [128, J1 * M4])
        hT_sb = sb("hT_sb", [128, C1 * M])
        u1T_sb = sb("u1T_sb", [128, C1 * M])
        hTs_sb = sb("hTs_sb", [128, J1 * M])
        u1Ts_sb = sb("u1Ts_sb", [128, J1 * M])
        u2Ts_sb = sb("u2Ts_sb", [128, J1 * M])
        pTs_sb = sb("pTs_sb", [128, J1 * M])
        qT_sb = sb("qT_sb", [128, C2 * M])
        v1T_sb = sb("v1T_sb", [128, C2 * M])
        qTs_sb = sb("qTs_sb", [128, J2 * M])
        v1Ts_sb = sb("v1Ts_sb", [128, J2 * M])
        v2Ts_sb = sb("v2Ts_sb", [128, J2 * M])
        idx_sb = sb("idx_sb", [128, J2 * DEG], I32)
        g_sb = sb("g_sb", [128, DEG * M])
        zc_sb = sb("zc_sb", [128, 12])
        tr_sb = sb("tr_sb", [64, 3 * 128])
        w_sb = sb("w_sb", [64, 128])
        a_sb = sb("a_sb", [64, 128])
        y_sb = sb("y_sb", [8, J2 * 128])
        w10cat_sb = sb("w10cat_sb", [12, 64])
        w11r_sb = sb("w11r_sb", [64, 192])
        w20r_sb = sb("w20r_sb", [64, 192])
        r0wr_sb = sb("r0wr_sb", [64, 64])
        r1wr_sb = sb("r1wr_sb", [64, 8])
        bvec_sb = sb("bvec_sb", [64, 8])
        ident_sb = sb("ident_sb", [128, 128])

        # 8 PSUM banks: one accumulation group per bank (start=True clears
        # has_written for the WHOLE bank, so groups must not share banks).
        pmb = [psa(f"pmb{j}", [128, 512]) for j in range(8)]
        # combine-phase psum aliases banks 4..7 (safe: sem chains order all
        # combine writes strictly after the epilogue copies of those banks,
        # and strictly before the next pass's matmuls)
        pt = pmb[4]
        pc = pmb[5]
        pt2 = pmb[6]
        py = pmb[7]

        s_dma = mksem("s_dma")
        s_aux = mksem("s_aux")
        pe_free = mksem("pe_free")
        cc = mksem("cc")
        sem_of = {"pe": mksem("pe_c"), "dv": mksem("dv"),
                  "ac": mksem("ac"), "gp": mksem("gp")}

        # ---------------- pass schedule -----------------------------------
        passes = [
            ("A", C1, J1, M4, xT_sb, s1s, SLAB1),
            ("B", C1, J1, M4, z1T_sb, s1s, SLAB1),
            ("C", C1, J1, M, hT_sb, s1s, SLAB1),
            ("D", C1, J1, M, u1T_sb, s1s, SLAB1),
            ("E", C2, J2, M, qT_sb, s2s, SLAB2),
            ("F", C2, J2, M, v1T_sb, s2s, SLAB2),
        ]
        base = {}
        g = 0
        for p in passes:
            base[p[0]] = g
            g += p[1]
        # pe_free value when pass completes:
        pass_done_val = {p[0]: base[p[0]] + p[1] for p in passes}

        PRELOAD = 10 * 16
        rhs_ready = {"A": PRELOAD, "B": PRELOAD + 16, "C": PRELOAD + 32,
                     "D": PRELOAD + 48, "E": PRELOAD + 64, "F": PRELOAD + 80}
        # cc values: z1=1, h=2, u1=3, p=4, q=5, v1=6
        reload_spec = {"B": (z1_out, z1T_sb, M4, 1), "C": (h_out, hT_sb, M, 2),
                       "D": (u1_out, u1T_sb, M, 3), "E": (q_out, qT_sb, M, 5),
                       "F": (v1_out, v1T_sb, M, 6)}

        # ---------------- framework ops ------------------------------------
        ops = []

        def add(engine, fn, deps=(), extra_waits=(), inc=1, is_async=False,
                stage=0):
            op = Op(engine, fn, deps, extra_waits, inc, is_async, stage)
            ops.append(op)
            return op

        AGOP = mybir.AluOpType.bypass
        Relu = mybir.ActivationFunctionType.Relu
        Ident = mybir.ActivationFunctionType.Identity
        Max = mybir.AluOpType.max

        # ============ epilogue A + AG z1 ============
        def copy_banks(dest_sb, nj, m):
            def fn(e):
                ins = None
                for j in range(nj):
                    ins = e.tensor_copy(dest_sb[:, j * m:(j + 1) * m],
                                        pmb[j][:, 0:m])
                return ins
            return fn

        e_a = add("dv", copy_banks(z1Ts_sb, J1, M4),
                  extra_waits=[(pe_free, pass_done_val["A"])])
        d_a = add("gp", lambda e: e.dma_start(
            out=z1_in[:, :].rearrange("(j p) m -> p j m", p=128),
            in_=z1Ts_sb[:, :].rearrange("p (j m) -> p j m", m=M4)),
            deps=[e_a], inc=16, is_async=True)
        add("gp", lambda e: e.collective_compute(
            "AllGather", AGOP, replica_groups=RG,
            ins=[z1_in[:, :]], outs=[z1_out[:, :]]),
            deps=[d_a], inc=("cc", 1), is_async=True)

        # ============ epilogue B + layer-1 combine + AG h ============
        e_b = add("dv", copy_banks(z2Ts_sb, J1, M4),
                  extra_waits=[(pe_free, pass_done_val["B"])])
        p_t1 = p_c2 = p_m1 = p_a1 = p_t2 = p_c3 = None
        for jj in range(J1):
            s4 = slice(jj * M4, (jj + 1) * M4)

            def d1fn(e, s4=s4):
                e.tensor_copy(zc_sb[:, 0:4], xTs_sb[:, s4])
                e.tensor_copy(zc_sb[:, 4:8], z1Ts_sb[:, s4])
                return e.tensor_copy(zc_sb[:, 8:12], z2Ts_sb[:, s4])
            d1 = add("dv", d1fn, deps=[e_b, p_t1])
            t1 = add("pe", lambda e: e.transpose(
                pt[0:12, 0:128], zc_sb[:, :], ident_sb[:, :]),
                deps=[d1, p_c2], stage=1)
            c2 = add("dv", lambda e: e.tensor_copy(
                tr_sb[0:12, 0:128], pt[0:12, 0:128]), deps=[t1, p_m1])
            m1 = add("pe", lambda e: e.matmul(
                pc[0:64, 0:128], w10cat_sb[:, :], tr_sb[0:12, 0:128],
                start=True, stop=True, skip_group_check=True),
                deps=[c2, p_a1], stage=1)
            a1 = add("ac", lambda e: e.activation(
                w_sb[:, :], pc[0:64, 0:128], Relu, bias=bvec_sb[:, 0:1]),
                deps=[m1, p_t2])
            t2 = add("pe", lambda e: e.transpose(
                pt2[:, 0:64], w_sb[:, :], ident_sb[0:64, 0:64]),
                deps=[a1, p_c3], stage=1)
            c3 = add("dv", lambda e, jj=jj: e.tensor_copy(
                hTs_sb[:, jj * M:(jj + 1) * M], pt2[:, 0:64]), deps=[t2])
            p_t1, p_c2, p_m1, p_a1, p_t2, p_c3 = t1, c2, m1, a1, t2, c3
        d_b = add("gp", lambda e: e.dma_start(
            out=h_in[:, :].rearrange("(j p) m -> p j m", p=128),
            in_=hTs_sb[:, :].rearrange("p (j m) -> p j m", m=M)),
            deps=[p_c3], inc=16, is_async=True)
        add("gp", lambda e: e.collective_compute(
            "AllGather", AGOP, replica_groups=RG,
            ins=[h_in[:, :]], outs=[h_out[:, :]]),
            deps=[d_b], inc=("cc", 1), is_async=True)

        # ============ epilogue C + AG u1 ============
        e_c = add("dv", copy_banks(u1Ts_sb, J1, M),
                  extra_waits=[(pe_free, pass_done_val["C"])])
        d_c = add("gp", lambda e: e.dma_start(
            out=u1_in[:, :].rearrange("(j p) m -> p j m", p=128),
            in_=u1Ts_sb[:, :].rearrange("p (j m) -> p j m", m=M)),
            deps=[e_c], inc=16, is_async=True)
        add("gp", lambda e: e.collective_compute(
            "AllGather", AGOP, replica_groups=RG,
            ins=[u1_in[:, :]], outs=[u1_out[:, :]]),
            deps=[d_c], inc=("cc", 1), is_async=True)

        # ============ epilogue D + layer-2 combine + AG p ============
        e_d = add("dv", copy_banks(u2Ts_sb, J1, M),
                  extra_waits=[(pe_free, pass_done_val["D"])])
        p_lastcopy = p_m1 = p_a1 = p_t4 = p_c4 = None
        for jj in range(J1):
            sM = slice(jj * M, (jj + 1) * M)
            srcs = (hTs_sb, u1Ts_sb, u2Ts_sb)
            cks = []
            for k in range(3):
                war = cks[k - 1] if k > 0 else p_lastcopy
                tk = add("pe", lambda e, k=k, sM=sM, srcs=srcs: e.transpose(
                    pt[0:64, 0:128], srcs[k][:, sM], ident_sb[:, :]),
                    deps=[e_d, war], stage=2)
                ck = add("dv", lambda e, k=k: e.tensor_copy(
                    tr_sb[:, k * 128:(k + 1) * 128], pt[0:64, 0:128]),
                    deps=[tk, p_m1 if k == 0 else None])
                cks.append(ck)
            p_lastcopy = cks[2]

            def m1fn(e):
                e.matmul(pc[0:64, 0:128], w11r_sb[:, 0:64], tr_sb[:, 0:128],
                         start=True, stop=False, skip_group_check=True)
                e.matmul(pc[0:64, 0:128], w11r_sb[:, 64:128],
                         tr_sb[:, 128:256], start=False, stop=False,
                         skip_group_check=True)
                return e.matmul(pc[0:64, 0:128], w11r_sb[:, 128:192],
                                tr_sb[:, 256:384], start=False, stop=True,
                                skip_group_check=True)
            m1 = add("pe", m1fn, deps=[cks[0], cks[1], cks[2], p_a1], stage=2)
            a1 = add("ac", lambda e: e.activation(
                w_sb[:, :], pc[0:64, 0:128], Relu, bias=bvec_sb[:, 1:2]),
                deps=[m1, p_t4])
            t4 = add("pe", lambda e: e.transpose(
                pt2[:, 0:64], w_sb[:, :], ident_sb[0:64, 0:64]),
                deps=[a1, p_c4], stage=2)
            c4 = add("dv", lambda e, jj=jj: e.tensor_copy(
                pTs_sb[:, jj * M:(jj + 1) * M], pt2[:, 0:64]), deps=[t4])
            p_m1, p_a1, p_t4, p_c4 = m1, a1, t4, c4
        d_d = add("gp", lambda e: e.dma_start(
            out=p_in[:, :].rearrange("(j p) m -> p j m", p=128),
            in_=pTs_sb[:, :].rearrange("p (j m) -> p j m", m=M)),
            deps=[p_c4], inc=16, is_async=True)
        ag_p = add("gp", lambda e: e.collective_compute(
            "AllGather", AGOP, replica_groups=RG,
            ins=[p_in[:, :]], outs=[p_out[:, :]]),
            deps=[d_d], inc=("cc", 1), is_async=True)

        # ============ pooling ============
        p_mx = None
        for mc in range(J2):
            gg = []
            for d in range(DEG):
                col = mc * DEG + d
                ggd = add("gp", lambda e, d=d, col=col: e.indirect_dma_start(
                    out=g_sb[:, d * M:(d + 1) * M],
                    out_offset=None,
                    in_=p_out[:, :],
                    in_offset=bass.IndirectOffsetOnAxis(
                        ap=idx_sb[:, col:col + 1], axis=0)),
                    deps=[ag_p, p_mx], inc=16, is_async=True)
                gg.append(ggd)

            def mxfn(e, mc=mc):
                e.tensor_tensor(g_sb[:, 0:M], g_sb[:, 0:M],
                                g_sb[:, M:2 * M], op=Max)
                e.tensor_tensor(g_sb[:, 0:M], g_sb[:, 0:M],
                                g_sb[:, 2 * M:3 * M], op=Max)
                e.tensor_tensor(g_sb[:, 0:M], g_sb[:, 0:M],
                                g_sb[:, 3 * M:4 * M], op=Max)
                return e.tensor_scalar_max(
                    qTs_sb[:, mc * M:(mc + 1) * M], g_sb[:, 0:M], 0.0)
            p_mx = add("dv", mxfn, deps=gg)
        d_q = add("gp", lambda e: e.dma_start(
            out=q_in[:, :].rearrange("(j p) m -> p j m", p=128),
            in_=qTs_sb[:, :].rearrange("p (j m) -> p j m", m=M)),
            deps=[p_mx], inc=16, is_async=True)
        add("gp", lambda e: e.collective_compute(
            "AllGather", AGOP, replica_groups=RG,
            ins=[q_in[:, :]], outs=[q_out[:, :]]),
            deps=[d_q], inc=("cc", 1), is_async=True)

        # ============ epilogue E + AG v1 ============
        e_e = add("dv", copy_banks(v1Ts_sb, J2, M),
                  extra_waits=[(pe_free, pass_done_val["E"])])
        d_e = add("gp", lambda e: e.dma_start(
            out=v1_in[:, :].rearrange("(j p) m -> p j m", p=128),
            in_=v1Ts_sb[:, :].rearrange("p (j m) -> p j m", m=M)),
            deps=[e_e], inc=16, is_async=True)
        add("gp", lambda e: e.collective_compute(
            "AllGather", AGOP, replica_groups=RG,
            ins=[v1_in[:, :]], outs=[v1_out[:, :]]),
            deps=[d_e], inc=("cc", 1), is_async=True)

        # ============ epilogue F + stage-2 combine + readout ============
        e_f = add("dv", copy_banks(v2Ts_sb, J2, M),
                  extra_waits=[(pe_free, pass_done_val["F"])])
        p_lastcopy = p_m1 = p_a1 = p_m2 = p_a2 = p_m3 = p_a3 = None
        a3_ops = []
        for jj in range(J2):
            sM = slice(jj * M, (jj + 1) * M)
            srcs = (qTs_sb, v1Ts_sb, v2Ts_sb)
            cks = []
            for k in range(3):
                war = cks[k - 1] if k > 0 else p_lastcopy
                tk = add("pe", lambda e, k=k, sM=sM, srcs=srcs: e.transpose(
                    pt[0:64, 0:128], srcs[k][:, sM], ident_sb[:, :]),
                    deps=[e_f, war], stage=3)
                ck = add("dv", lambda e, k=k: e.tensor_copy(
                    tr_sb[:, k * 128:(k + 1) * 128], pt[0:64, 0:128]),
                    deps=[tk, p_m1 if k == 0 else None])
                cks.append(ck)
            p_lastcopy = cks[2]

            def m1fn(e):
                e.matmul(pc[0:64, 0:128], w20r_sb[:, 0:64], tr_sb[:, 0:128],
                         start=True, stop=False, skip_group_check=True)
                e.matmul(pc[0:64, 0:128], w20r_sb[:, 64:128],
                         tr_sb[:, 128:256], start=False, stop=False,
                         skip_group_check=True)
                return e.matmul(pc[0:64, 0:128], w20r_sb[:, 128:192],
                                tr_sb[:, 256:384], start=False, stop=True,
                                skip_group_check=True)
            m1 = add("pe", m1fn, deps=[cks[0], cks[1], cks[2], p_a1],
                     stage=3)
            a1 = add("ac", lambda e: e.activation(
                w_sb[:, :], pc[0:64, 0:128], Relu, bias=bvec_sb[:, 2:3]),
                deps=[m1, p_m2])
            m2 = add("pe", lambda e: e.matmul(
                pc[0:64, 0:128], r0wr_sb[:, :], w_sb[:, :],
                start=True, stop=True, skip_group_check=True),
                deps=[a1, p_a2], stage=3)
            a2 = add("ac", lambda e: e.activation(
                a_sb[:, :], pc[0:64, 0:128], Relu, bias=bvec_sb[:, 3:4]),
                deps=[m2, p_m3])
            m3 = add("pe", lambda e: e.matmul(
                py[0:8, 0:128], r1wr_sb[:, :], a_sb[:, :],
                start=True, stop=True, skip_group_check=True),
                deps=[a2, p_a3], stage=3)
            a3 = add("ac", lambda e, jj=jj: e.activation(
                y_sb[:, jj * 128:(jj + 1) * 128], py[0:8, 0:128], Ident,
                bias=bvec_sb[0:8, 4:5]), deps=[m3])
            p_m1, p_a1, p_m2, p_a2, p_m3, p_a3 = m1, a1, m2, a2, m3, a3
            a3_ops.append(a3)
        y_dma = add("gp", lambda e: e.dma_start(out=y_d[:, :], in_=y_sb[:, :]),
                    deps=list(a3_ops), inc=16, is_async=True)

        if DEBUG:
            dbg_specs = [
                ("dbg_z1", z1_out, [N1, M4]), ("dbg_h", h_out, [N1, M]),
                ("dbg_u1", u1_out, [N1, M]), ("dbg_p", p_out, [N1, M]),
                ("dbg_q", q_out, [N2, M]), ("dbg_v1", v1_out, [N2, M]),
            ]
            for (nm, src_t, shp) in dbg_specs:
                dst = nc.declare_dram_parameter(nm, shp, F32, isOutput=True)
                add("gp", lambda e, dst=dst, src_t=src_t: e.dma_start(
                    out=dst[:, :], in_=src_t[:, :]),
                    deps=[y_dma], extra_waits=[(cc, 6)], inc=16,
                    is_async=True)

        # ---------------- assign sem values --------------------------------
        counters = {"pe": 0, "dv": 0, "ac": 0, "gp": 0, "cc": 0}
        for op in ops:
            if isinstance(op.inc, tuple):
                counters["cc"] += op.inc[1]
                op.sem = cc
                op.val = counters["cc"]
            else:
                counters[op.engine] += op.inc
                op.sem = sem_of[op.engine]
                op.val = counters[op.engine]
        gp_final = counters["gp"]

        def emit_ops(engine_key, e, stage=None):
            for op in ops:
                if op.engine != engine_key:
                    continue
                if engine_key == "pe" and stage is not None and \
                        op.stage != stage:
                    continue
                for dep in op.deps:
                    if dep.engine == engine_key and not dep.is_async:
                        continue
                    e.wait_ge(dep.sem, dep.val)
                for (s, v) in op.extra_waits:
                    e.wait_ge(s, v)
                ins = op.fn(e)
                if isinstance(op.inc, tuple):
                    ins.then_inc(op.sem, op.inc[1])
                else:
                    ins.then_inc(op.sem, op.inc)

        pe_stage_after = {"B": 1, "D": 2, "F": 3}

        with nc.Block() as block:

            @block.sync
            def _(sync):
                for (dst, src) in [
                        (w10cat_sb, w10cat_d), (w11r_sb, w11r_d),
                        (w20r_sb, w20r_d), (r0wr_sb, r0wr_d),
                        (r1wr_sb, r1wr_d), (bvec_sb, bvec_d),
                        (ident_sb, ident_d)]:
                    sync.dma_start(out=dst[:, :], in_=src[:, :]).then_inc(
                        s_aux, 16)
                sync.dma_start(
                    out=xT_sb[:, :].rearrange("p (i m) -> p i m", m=M4),
                    in_=xT_d[:, :].rearrange("(i p) m -> p i m", p=128),
                ).then_inc(s_aux, 16)
                sync.dma_start(
                    out=xTs_sb[:, :].rearrange("p (i m) -> p i m", m=M4),
                    in_=xTs_d[:, :].rearrange("(i p) m -> p i m", p=128),
                ).then_inc(s_aux, 16)
                sync.dma_start(
                    out=idx_sb[:, :].rearrange("p (i m) -> p i m", m=DEG),
                    in_=incs_d[:, :].rearrange("(i p) m -> p i m", p=128),
                ).then_inc(s_aux, 16)

                for (name, n_chunks, j_tiles, m, rhs, src, w) in passes:

                    def chunk_dma(i1, name=name, src=src, w=w):
                        gci = base[name] + i1
                        if gci >= NBUF:
                            sync.wait_ge(pe_free, gci - NBUF + 1)
                        slot = gci % NBUF
                        sync.dma_start(
                            out=sring[:, slot * SLAB1: slot * SLAB1 + w],
                            in_=src[i1 * 128:(i1 + 1) * 128, :],
                        ).then_inc(s_dma, 16)

                    npre = min(NBUF, n_chunks)
                    for i1 in range(npre):
                        chunk_dma(i1)
                    if name in reload_spec:
                        cc_src, dest_sb, m_r, cc_val = reload_spec[name]
                        sync.wait_ge(cc, cc_val)
                        sync.dma_start(
                            out=dest_sb[:, :].rearrange(
                                "p (i m) -> p i m", m=m_r),
                            in_=cc_src[:, :].rearrange(
                                "(i p) m -> p i m", p=128),
                        ).then_inc(s_aux, 16)
                    for i1 in range(npre, n_chunks):
                        chunk_dma(i1)

            @block.tensor
            def _(tensor):
                for (name, n_chunks, j_tiles, m, rhs, src, w) in passes:
                    for i1 in range(n_chunks):
                        gci = base[name] + i1
                        tensor.wait_ge(s_dma, (gci + 1) * 16)
                        if i1 == 0:
                            tensor.wait_ge(s_aux, rhs_ready[name])
                        slot = gci % NBUF
                        ins = None
                        for j in range(j_tiles):
                            ins = tensor.matmul(
                                pmb[j][:, 0:m],
                                sring[:, slot * SLAB1 + j * 128:
                                      slot * SLAB1 + (j + 1) * 128],
                                rhs[:, i1 * m:(i1 + 1) * m],
                                start=(i1 == 0),
                                stop=(i1 == n_chunks - 1),
                                skip_group_check=True,
                            )
                        ins.then_inc(pe_free, 1)
                    if name in pe_stage_after:
                        emit_ops("pe", tensor, stage=pe_stage_after[name])

            @block.vector
            def _(vector):
                emit_ops("dv", vector)

            @block.scalar
            def _(scalar):
                emit_ops("ac", scalar)

            @block.gpsimd
            def _(gpsimd):
                emit_ops("gp", gpsimd)
                gpsimd.wait_ge(sem_of["gp"], gp_final)

    return nc


def _get_nc():
    if "nc" not in _CACHE:
        _CACHE["nc"] = _build_nc()
    return _CACHE["nc"]


def kernel(x, S1, S2, incid_idx, W10, b10, W11, b11, W20, b20,
           R0W, R0b, R1W, R1b, _results_out=None, **run_kwargs):
    in_maps = _host_prep(x, S1, S2, incid_idx, W10, W11, W20,
                         b10, b11, b20, R0W, R0b, R1W, R1b)
    nc = _get_nc()
    res = run_bass_kernel_spmd(nc, in_maps, CORE_IDS, **run_kwargs)
    if _results_out is not None:
        _results_out.append(res)
    y = np.zeros((B, 2, N2), np.float32)
    for c in range(NC):
        yc = res.results[c]["y"]                      # [8, SLAB2]
        y[:, :, c * SLAB2:(c + 1) * SLAB2] = yc.reshape(B, 2, SLAB2)
    return y


# revision 14
# speedup vs baseline: 2.6312x; 2.6312x over previous
"""LocalGNNCliqueLine Trainium2 kernel, v2: bf16, streamed-S orientation,
S1 slab resident in SBUF.

Reference (B=4, N1=8192, N2=4096, DEG=4, F0=1, F1=F2=16, K=3, R0=16, R1=2):
  h1 = relu(gfilter(x, S1, W10, b10))          # [B,16,N1]
  h2 = relu(gfilter(h1, S1, W11, b11))         # [B,16,N1]
  q  = relu-max-pool(h2, incid_idx)            # [B,16,N2]
  h3 = relu(gfilter(q, S2, W20, b20))          # [B,16,N2]
  y  = (relu(h3^T @ R0W + R0b) @ R1W + R1b)^T  # [B,2,N2]

Sharding: S1/S2 column(=output node)-sharded across 8 cores (slabs cast to
bf16 on the host). Each GSO tap out[m, n2_slab] = z @ S_slab runs with the
small z^T chunk as the PE stationary operand and S chunks as the N=512
moving operand (fp32 PSUM accumulation). The S1 slab is streamed once
during tap 1 into a resident SBUF buffer; taps 2-4 run from SBUF with no
DMA. Slab outputs are PE-transposed back to node-on-partition layout,
AllGathered in bf16, and chained. Feature mixing, pooling (indirect-DMA
gather) and the readout MLP run on slab rows via PE transpose + small
block-diagonal matmuls with ping-pong PSUM banks.
"""

import numpy as np
import ml_dtypes

import concourse.bass as bass
import concourse.mybir as mybir
from concourse.bass_utils import run_bass_kernel_spmd

F32 = mybir.dt.float32
BF16 = mybir.dt.bfloat16
I32 = mybir.dt.int32
NPBF = ml_dtypes.bfloat16

NC = 8
CORE_IDS = list(range(NC))
B, N1, N2, DEG = 4, 8192, 4096, 4
F1 = 16
M4 = B                       # 4 stacked rows for layer-1 taps (F0=1)
M = B * F1                   # 64
SLAB1 = N1 // NC             # 1024
SLAB2 = N2 // NC             # 512
C1 = N1 // 128               # 64
C2 = N2 // 128               # 32
J1 = SLAB1 // 128            # 8
J2 = SLAB2 // 128            # 4
NB2 = 8                      # S2 streaming ring depth

RG = [CORE_IDS]
_CACHE = {}
DEBUG = False


def _host_prep(x, S1, S2, incid_idx, W10, W11, W20, b10, b11, b20,
               R0W, R0b, R1W, R1b):
    f = np.float32
    xT = np.asarray(x, f)[:, 0, :].T                               # [N1, 4]

    w10cat = np.zeros((12, 64), f)                                 # rows (k,b)
    for k in range(3):
        for b in range(4):
            w10cat[k * 4 + b, b * 16:(b + 1) * 16] = np.asarray(W10, f)[k, 0, :]

    def blockdiag3(W):   # [3,16,16] -> [64, 192]; rows (b,f), k-major cols (b,g)
        out = np.zeros((64, 192), f)
        for k in range(3):
            for b in range(4):
                out[b * 16:(b + 1) * 16,
                    k * 64 + b * 16:k * 64 + (b + 1) * 16] = np.asarray(W, f)[k]
        return out

    def blockdiag1(W):   # [16, r] -> [64, 4r]
        W = np.asarray(W, f)
        r = W.shape[1]
        out = np.zeros((64, 4 * r), f)
        for b in range(4):
            out[b * 16:(b + 1) * 16, b * r:(b + 1) * r] = W
        return out

    bvec = np.zeros((64, 8), f)
    bvec[:, 0] = np.tile(np.asarray(b10, f), 4)
    bvec[:, 1] = np.tile(np.asarray(b11, f), 4)
    bvec[:, 2] = np.tile(np.asarray(b20, f), 4)
    bvec[:, 3] = np.tile(np.asarray(R0b, f), 4)
    bvec[:8, 4] = np.tile(np.asarray(R1b, f), 4)

    bf = lambda a: np.ascontiguousarray(np.asarray(a, f).astype(NPBF))  # noqa: E731
    S1 = np.asarray(S1, f)
    S2 = np.asarray(S2, f)
    incid = np.asarray(incid_idx, np.int32)

    common = {
        "xT": bf(xT),
        "w10cat": bf(w10cat), "w11r": bf(blockdiag3(W11)),
        "w20r": bf(blockdiag3(W20)), "r0wr": bf(blockdiag1(R0W)),
        "r1wr": bf(blockdiag1(R1W)), "bvec": np.ascontiguousarray(bvec),
        "ident": bf(np.eye(128, dtype=f)),
    }
    in_maps = []
    for c in range(NC):
        m = dict(common)
        m["s1s"] = bf(S1[:, c * SLAB1:(c + 1) * SLAB1])
        m["s2s"] = bf(S2[:, c * SLAB2:(c + 1) * SLAB2])
        m["xTs"] = bf(xT[c * SLAB1:(c + 1) * SLAB1])
        m["incs"] = np.ascontiguousarray(incid[c * SLAB2:(c + 1) * SLAB2])
        in_maps.append(m)
    return in_maps


class Op:
    __slots__ = ("engine", "fn", "deps", "extra_waits", "inc", "is_async",
                 "stage", "sem", "val")

    def __init__(self, engine, fn, deps=(), extra_waits=(), inc=1,
                 is_async=False, stage=0):
        self.engine = engine
        self.fn = fn
        self.deps = [d for d in deps if d is not None]
        self.extra_waits = list(extra_waits)
        self.inc = inc
        self.is_async = is_async
        self.stage = stage
        self.sem = None
        self.val = None


def _build_nc():
    nc = bass.Bass(num_devices=NC)

    s1s = nc.declare_dram_parameter("s1s", [N1, SLAB1], BF16, isOutput=False)
    s2s = nc.declare_dram_parameter("s2s", [N2, SLAB2], BF16, isOutput=False)
    xT_d = nc.declare_dram_parameter("xT", [N1, M4], BF16, isOutput=False)
    xTs_d = nc.declare_dram_parameter("xTs", [SLAB1, M4], BF16, isOutput=False)
    incs_d = nc.declare_dram_parameter("incs", [SLAB2, DEG], I32, isOutput=False)
    w10cat_d = nc.declare_dram_parameter("w10cat", [12, 64], BF16, isOutput=False)
    w11r_d = nc.declare_dram_parameter("w11r", [64, 192], BF16, isOutput=False)
    w20r_d = nc.declare_dram_parameter("w20r", [64, 192], BF16, isOutput=False)
    r0wr_d = nc.declare_dram_parameter("r0wr", [64, 64], BF16, isOutput=False)
    r1wr_d = nc.declare_dram_parameter("r1wr", [64, 8], BF16, isOutput=False)
    bvec_d = nc.declare_dram_parameter("bvec", [64, 8], F32, isOutput=False)
    ident_d = nc.declare_dram_parameter("ident", [128, 128], BF16, isOutput=False)
    y_d = nc.declare_dram_parameter("y", [8, SLAB2], F32, isOutput=True)

    z1_in = nc.dram_tensor("z1_in", [SLAB1, M4], BF16)
    z1_out = nc.dram_tensor("z1_out", [N1, M4], BF16, addr_space="Shared")
    h_in = nc.dram_tensor("h_in", [SLAB1, M], BF16)
    h_out = nc.dram_tensor("h_out", [N1, M], BF16, addr_space="Shared")
    u1_in = nc.dram_tensor("u1_in", [SLAB1, M], BF16)
    u1_out = nc.dram_tensor("u1_out", [N1, M], BF16, addr_space="Shared")
    p_in = nc.dram_tensor("p_in", [SLAB1, M], BF16)
    p_out = nc.dram_tensor("p_out", [N1, M], BF16, addr_space="Shared")
    q_in = nc.dram_tensor("q_in", [SLAB2, M], BF16)
    q_out = nc.dram_tensor("q_out", [N2, M], BF16, addr_space="Shared")
    v1_in = nc.dram_tensor("v1_in", [SLAB2, M], BF16)
    v1_out = nc.dram_tensor("v1_out", [N2, M], BF16, addr_space="Shared")

    from contextlib import ExitStack
    ctx = ExitStack()
    sb = lambda name, shape, dt=BF16: ctx.enter_context(  # noqa: E731
        nc.sbuf_tensor(name, shape, dt))
    psa = lambda name, dt=F32: ctx.enter_context(  # noqa: E731
        nc.psum_tensor(name, [128, 512], dt))
    mksem = lambda name: ctx.enter_context(nc.semaphore(name))  # noqa: E731

    with ctx:
        s1r = sb("s1r", [128, C1 * SLAB1])            # resident S1 slab bf16
        s2ring = sb("s2ring", [128, NB2 * SLAB2])     # S2 stream ring
        xT_sb = sb("xT_sb", [128, C1 * M4])
        z1T_sb = sb("z1T_sb", [128, C1 * M4])
        xTs_sb = sb("xTs_sb", [128, J1 * M4])
        z1Ts_sb = sb("z1Ts_sb", [128, J1 * M4])
        z2Ts_sb = sb("z2Ts_sb", [128, J1 * M4])
        hT_sb = sb("hT_sb", [128, C1 * M])
        u1T_sb = sb("u1T_sb", [128, C1 * M])
        hTs_sb = sb("hTs_sb", [128, J1 * M])
        u1Ts_sb = sb("u1Ts_sb", [128, J1 * M])
        u2Ts_sb = sb("u2Ts_sb", [128, J1 * M])
        pTs_sb = sb("pTs_sb", [128, J1 * M])
        qT_sb = sb("qT_sb", [128, C2 * M])
        v1T_sb = sb("v1T_sb", [128, C2 * M])
        qTs_sb = sb("qTs_sb", [128, J2 * M])
        v1Ts_sb = sb("v1Ts_sb", [128, J2 * M])
        v2Ts_sb = sb("v2Ts_sb", [128, J2 * M])
        idx_sb = sb("idx_sb", [128, J2 * DEG], I32)
        g_sb = sb("g_sb", [128, (DEG + 2) * M])
        pout_sb = sb("pout_sb", [64, 1024])           # pass output staging
        zc_sb = sb("zc_sb", [128, 2 * 12])            # parity halves
        tr_sb = sb("tr_sb", [64, 2 * 384])
        w_sb = sb("w_sb", [64, 2 * 128])
        a_sb = sb("a_sb", [64, 2 * 128])
        y_sb = sb("y_sb", [8, J2 * 128], F32)
        w10cat_sb = sb("w10cat_sb", [12, 64])
        w11r_sb = sb("w11r_sb", [64, 192])
        w20r_sb = sb("w20r_sb", [64, 192])
        r0wr_sb = sb("r0wr_sb", [64, 64])
        r1wr_sb = sb("r1wr_sb", [64, 8])
        bvec_sb = sb("bvec_sb", [64, 8], F32)
        ident_sb = sb("ident_sb", [128, 128])

        p0 = psa("p0")        # pass accumulator, slab cols 0:512
        p1 = psa("p1")        # pass accumulator, slab cols 512:1024
        ptrb = [psa("ptr0", BF16), psa("ptr1", BF16)]  # epilogue T ping-pong
        ptp = [psa("ptA", BF16), psa("ptB", BF16)]  # combine transposes
        pcp = [psa("pcA"), psa("pcB")]    # combine matmul outputs (f32)

        NSD = 8
        s_dmaN = [mksem(f"s_dma{i}") for i in range(NSD)]
        s_aux = mksem("s_aux")
        pe_free = mksem("pe_free")
        cc = mksem("cc")
        sem_of = {"pe": mksem("pe_c"), "dv": mksem("dv"),
                  "ac": mksem("ac"), "gp": mksem("gp")}

        # ---------------- schedule ----------------------------------------
        # (name, n_chunks, m, lhsT_sb, streamed, n_mm, stage)
        passes = [
            ("A", C1, M4, xT_sb, True, 2, 1),
            ("B", C1, M4, z1T_sb, False, 2, 2),
            ("C", C1, M, hT_sb, False, 2, 3),
            ("D", C1, M, u1T_sb, False, 2, 4),
            ("E", C2, M, qT_sb, True, 1, 5),
            ("F", C2, M, v1T_sb, True, 1, 6),
        ]
        pe_done_val = {}
        v = 0
        for p in passes:
            v += p[1]
            pe_done_val[p[0]] = v

        PRELOAD = 10 * 16
        rhs_ready = {"A": PRELOAD, "B": PRELOAD + 16, "C": PRELOAD + 32,
                     "D": PRELOAD + 48, "E": PRELOAD + 64, "F": PRELOAD + 80}
        reload_spec = {"B": (z1_out, z1T_sb, M4, 1), "C": (h_out, hT_sb, M, 2),
                       "D": (u1_out, u1T_sb, M, 3), "E": (q_out, qT_sb, M, 5),
                       "F": (v1_out, v1T_sb, M, 6)}

        ops = []

        def add(engine, fn, deps=(), extra_waits=(), inc=1, is_async=False,
                stage=0):
            op = Op(engine, fn, deps, extra_waits, inc, is_async, stage)
            ops.append(op)
            return op

        AGOP = mybir.AluOpType.bypass
        Relu = mybir.ActivationFunctionType.Relu
        Ident = mybir.ActivationFunctionType.Identity
        Max = mybir.AluOpType.max

        # ---------------- pass epilogues -----------------------------------
        # copy psum -> pout (bf16), PE-transpose slab chunks back to
        # node-on-partition, copy into the slab tensor.
        prev_tp = {}    # j -> last transpose op (WAR for ptr slot j)
        prev_cp = {}    # j -> last slot copy op
        last_tp_of_pass = {}

        def epilogue(name, m, nj, dest_slab, stage):
            def e1fn(e, m=m, nj=nj):
                ins = e.tensor_copy(pout_sb[0:m, 0:512], p0[0:m, 0:512])
                if nj > 4:
                    ins = e.tensor_copy(pout_sb[0:m, 512:1024],
                                        p1[0:m, 0:512])
                return ins
            e1 = add("dv", e1fn, extra_waits=[(pe_free, pe_done_val[name])])
            cps = []
            for j in range(nj):
                pX = ptrb[j % 2]
                # bank guard: PE may not write this ptr bank while the DVE
                # copy two steps back (same bank) might still be reading it
                tp = add("pe", lambda e, j=j, m=m, pX=pX: e.transpose(
                    pX[:, j * 64:j * 64 + m], pout_sb[0:m,
                                                      j * 128:(j + 1) * 128],
                    ident_sb[0:m, 0:m]),
                    deps=[e1, cps[j - 2] if j >= 2 else None],
                    stage=stage)
                cp = add("dv", lambda e, j=j, m=m, pX=pX: e.tensor_copy(
                    dest_slab[:, j * m:(j + 1) * m], pX[:, j * 64:j * 64 + m]),
                    deps=[tp])
                prev_tp[j] = tp
                prev_cp[j] = cp
                cps.append(cp)
            last_tp_of_pass[name] = cps
            return cps

        def stage_ag(name_in, name_out, src_slab, m, cc_deps):
            d = add("gp", lambda e: e.dma_start(
                out=name_in[:, :].rearrange("(j p) m -> p j m", p=128),
                in_=src_slab[:, :].rearrange("p (j m) -> p j m", m=m)),
                deps=cc_deps, inc=16, is_async=True)
            return add("gp", lambda e: e.collective_compute(
                "AllGather", AGOP, replica_groups=RG,
                ins=[name_in[:, :]], outs=[name_out[:, :]]),
                deps=[d], inc=("cc", 1), is_async=True)

        # pass A epilogue + AG z1
        cpsA = epilogue("A", M4, J1, z1Ts_sb, 1)
        stage_ag(z1_in, z1_out, z1Ts_sb, M4, cpsA)

        # pass B epilogue + layer-1 combine + AG h
        cpsB = epilogue("B", M4, J1, z2Ts_sb, 2)
        p_by = {}
        ccs = []
        for jj in range(J1):
            par = jj % 2
            ptX, pcX = ptp[par], pcp[par]
            zc = zc_sb[:, par * 12:par * 12 + 12]
            tr0 = tr_sb[0:12, par * 384:par * 384 + 128]
            wsb = w_sb[:, par * 128:(par + 1) * 128]
            s4 = slice(jj * M4, (jj + 1) * M4)

            def d1fn(e, s4=s4, zc=zc):
                e.tensor_copy(zc[:, 0:4], xTs_sb[:, s4])
                e.tensor_copy(zc[:, 4:8], z1Ts_sb[:, s4])
                return e.tensor_copy(zc[:, 8:12], z2Ts_sb[:, s4])
            d1 = add("dv", d1fn,
                     deps=[cpsA[jj], cpsB[jj], p_by.get(("t1", par))])
            t1 = add("pe", lambda e, zc=zc, ptX=ptX: e.transpose(
                ptX[0:12, 0:128], zc, ident_sb[:, :]),
                deps=[d1, p_by.get(("c3", par))], stage=2)
            c2 = add("dv", lambda e, ptX=ptX, tr0=tr0: e.tensor_copy(
                tr0, ptX[0:12, 0:128]),
                deps=[t1, p_by.get(("m1", par))])
            m1 = add("pe", lambda e, pcX=pcX, tr0=tr0: e.matmul(
                pcX[0:64, 0:128], w10cat_sb[:, :], tr0,
                start=True, stop=True, skip_group_check=True),
                deps=[c2, p_by.get(("a1", par))], stage=2)
            a1 = add("ac", lambda e, pcX=pcX, wsb=wsb: e.activation(
                wsb, pcX[0:64, 0:128], Relu, bias=bvec_sb[:, 0:1]),
                deps=[m1, p_by.get(("t2", par))])
            t2 = add("pe", lambda e, ptX=ptX, wsb=wsb: e.transpose(
                ptX[:, 128:192], wsb, ident_sb[0:64, 0:64]),
                deps=[a1], stage=2)
            c3 = add("dv", lambda e, jj=jj, ptX=ptX: e.tensor_copy(
                hTs_sb[:, jj * M:(jj + 1) * M], ptX[:, 128:192]),
                deps=[t2])
            for k, o in [("t1", t1), ("c3", c3), ("m1", m1), ("a1", a1),
                         ("t2", t2)]:
                p_by[(k, par)] = o
            ccs.append(c3)
        stage_ag(h_in, h_out, hTs_sb, M, ccs)

        # pass C epilogue + AG u1
        cpsC = epilogue("C", M, J1, u1Ts_sb, 3)
        stage_ag(u1_in, u1_out, u1Ts_sb, M, cpsC)

        # pass D epilogue + layer-2 combine + AG p
        cpsD = epilogue("D", M, J1, u2Ts_sb, 4)
        p_by = {}
        ccs = []
        for jj in range(J1):
            par = jj % 2
            ptX, pcX = ptp[par], pcp[par]
            wsb = w_sb[:, par * 128:(par + 1) * 128]
            sM = slice(jj * M, (jj + 1) * M)
            srcs = (hTs_sb, u1Ts_sb, u2Ts_sb)
            src_dep = (None, None, cpsD[jj])
            cks = []
            for k in range(3):
                war = cks[k - 1] if k > 0 else p_by.get(("c4", par))
                tk = add("pe", lambda e, k=k, sM=sM, srcs=srcs, ptX=ptX:
                         e.transpose(ptX[0:64, k * 128:(k + 1) * 128],
                                     srcs[k][:, sM], ident_sb[:, :]),
                         deps=[src_dep[k], war], stage=4)
                ck = add("dv", lambda e, k=k, par=par, ptX=ptX: e.tensor_copy(
                    tr_sb[:, par * 384 + k * 128:par * 384 + (k + 1) * 128],
                    ptX[0:64, k * 128:(k + 1) * 128]),
                    deps=[tk, p_by.get(("m1", par)) if k == 0 else None])
                cks.append(ck)

            def m1fn(e, par=par, pcX=pcX):
                t0 = par * 384
                e.matmul(pcX[0:64, 0:128], w11r_sb[:, 0:64],
                         tr_sb[:, t0:t0 + 128], start=True, stop=False,
                         skip_group_check=True)
                e.matmul(pcX[0:64, 0:128], w11r_sb[:, 64:128],
                         tr_sb[:, t0 + 128:t0 + 256], start=False, stop=False,
                         skip_group_check=True)
                return e.matmul(pcX[0:64, 0:128], w11r_sb[:, 128:192],
                                tr_sb[:, t0 + 256:t0 + 384], start=False,
                                stop=True, skip_group_check=True)
            m1 = add("pe", m1fn, deps=[cks[0], cks[1], cks[2],
                                       p_by.get(("a1", par))], stage=4)
            a1 = add("ac", lambda e, pcX=pcX, wsb=wsb: e.activation(
                wsb, pcX[0:64, 0:128], Relu, bias=bvec_sb[:, 1:2]),
                deps=[m1, p_by.get(("t4", par))])
            t4 = add("pe", lambda e, ptX=ptX, wsb=wsb: e.transpose(
                ptX[:, 384:448], wsb, ident_sb[0:64, 0:64]),
                deps=[a1], stage=4)
            c4 = add("dv", lambda e, jj=jj, ptX=ptX: e.tensor_copy(
                pTs_sb[:, jj * M:(jj + 1) * M], ptX[:, 384:448]),
                deps=[t4])
            for k, o in [("c4", c4), ("m1", m1), ("a1", a1), ("t4", t4)]:
                p_by[(k, par)] = o
            ccs.append(c4)
        ag_p = stage_ag(p_in, p_out, pTs_sb, M, ccs)

        # pooling
        p_mx = None
        mx_ops = []
        for mc in range(J2):
            gg = []
            for d in range(DEG):
                col = mc * DEG + d
                gg.append(add("gp", lambda e, d=d, col=col:
                              e.indirect_dma_start(
                                  out=g_sb[:, d * M:(d + 1) * M],
                                  out_offset=None,
                                  in_=p_out[:, :],
                                  in_offset=bass.IndirectOffsetOnAxis(
                                      ap=idx_sb[:, col:col + 1], axis=0)),
                              deps=[ag_p, p_mx], inc=16, is_async=True))

            def mxfn(e, mc=mc):
                e.tensor_tensor(g_sb[:, 4 * M:5 * M], g_sb[:, 0:M],
                                g_sb[:, M:2 * M], op=Max)
                e.tensor_tensor(g_sb[:, 5 * M:6 * M], g_sb[:, 2 * M:3 * M],
                                g_sb[:, 3 * M:4 * M], op=Max)
                e.drain()
                e.tensor_tensor(g_sb[:, 4 * M:5 * M], g_sb[:, 4 * M:5 * M],
                                g_sb[:, 5 * M:6 * M], op=Max)
                e.drain()
                return e.tensor_scalar_max(
                    qTs_sb[:, mc * M:(mc + 1) * M], g_sb[:, 4 * M:5 * M], 0.0)
            p_mx = add("dv", mxfn, deps=gg)
            mx_ops.append(p_mx)
        stage_ag(q_in, q_out, qTs_sb, M, [mx_ops[-1]])

        # pass E epilogue + AG v1
        cpsE = epilogue("E", M, J2, v1Ts_sb, 5)
        stage_ag(v1_in, v1_out, v1Ts_sb, M, cpsE)

        # pass F epilogue + stage-2 combine + readout
        cpsF = epilogue("F", M, J2, v2Ts_sb, 6)
        p_by = {}
        a3_ops = []
        for jj in range(J2):
            par = jj % 2
            ptX, pcX = ptp[par], pcp[par]
            wsb = w_sb[:, par * 128:(par + 1) * 128]
            asb = a_sb[:, par * 128:(par + 1) * 128]
            sM = slice(jj * M, (jj + 1) * M)
            srcs = (qTs_sb, v1Ts_sb, v2Ts_sb)
            src_dep = (mx_ops[jj], cpsE[jj], cpsF[jj])
            cks = []
            for k in range(3):
                war = cks[k - 1] if k > 0 else p_by.get(("c4", par))
                tk = add("pe", lambda e, k=k, sM=sM, srcs=srcs, ptX=ptX:
                         e.transpose(ptX[0:64, k * 128:(k + 1) * 128],
                                     srcs[k][:, sM], ident_sb[:, :]),
                         deps=[src_dep[k], war], stage=6)
                ck = add("dv", lambda e, k=k, par=par, ptX=ptX: e.tensor_copy(
                    tr_sb[:, par * 384 + k * 128:par * 384 + (k + 1) * 128],
                    ptX[0:64, k * 128:(k + 1) * 128]),
                    deps=[tk, p_by.get(("m1", par)) if k == 0 else None])
                cks.append(ck)
            # no back-transpose in stage 6; c4 key tracks last ptX reader
            p_by[("c4", par)] = cks[2]

            def m1fn(e, par=par, pcX=pcX):
                t0 = par * 384
                e.matmul(pcX[0:64, 0:128], w20r_sb[:, 0:64],
                         tr_sb[:, t0:t0 + 128], start=True, stop=False,
                         skip_group_check=True)
                e.matmul(pcX[0:64, 0:128], w20r_sb[:, 64:128],
                         tr_sb[:, t0 + 128:t0 + 256], start=False, stop=False,
                         skip_group_check=True)
                return e.matmul(pcX[0:64, 0:128], w20r_sb[:, 128:192],
                                tr_sb[:, t0 + 256:t0 + 384], start=False,
                                stop=True, skip_group_check=True)
            m1 = add("pe", m1fn, deps=[cks[0], cks[1], cks[2],
                                       p_by.get(("a3", par))], stage=6)
            a1 = add("ac", lambda e, pcX=pcX, wsb=wsb: e.activation(
                wsb, pcX[0:64, 0:128], Relu, bias=bvec_sb[:, 2:3]),
                deps=[m1, p_by.get(("m2", par))])
            m2 = add("pe", lambda e, pcX=pcX, wsb=wsb: e.matmul(
                pcX[0:64, 128:256], r0wr_sb[:, :], wsb,
                start=True, stop=True, skip_group_check=True),
                deps=[a1, p_by.get(("a2", par))], stage=6)
            a2 = add("ac", lambda e, pcX=pcX, asb=asb: e.activation(
                asb, pcX[0:64, 128:256], Relu, bias=bvec_sb[:, 3:4]),
                deps=[m2, p_by.get(("m3", par))])
            m3 = add("pe", lambda e, pcX=pcX, asb=asb: e.matmul(
                pcX[0:8, 256:384], r1wr_sb[:, :], asb,
                start=True, stop=True, skip_group_check=True),
                deps=[a2, p_by.get(("a3", par))], stage=6)
            a3 = add("ac", lambda e, jj=jj, pcX=pcX: e.activation(
                y_sb[:, jj * 128:(jj + 1) * 128], pcX[0:8, 256:384], Ident,
                bias=bvec_sb[0:8, 4:5]), deps=[m3])
            for k, o in [("m1", m1), ("m2", m2), ("m3", m3), ("a2", a2),
                         ("a3", a3)]:
                p_by[(k, par)] = o
            a3_ops.append(a3)
        y_dma = add("gp", lambda e: e.dma_start(out=y_d[:, :], in_=y_sb[:, :]),
                    deps=list(a3_ops), inc=16, is_async=True)

        if DEBUG:
            dbg_specs = [
                ("dbg_z1", z1_out, [N1, M4]), ("dbg_h", h_out, [N1, M]),
                ("dbg_u1", u1_out, [N1, M]), ("dbg_p", p_out, [N1, M]),
                ("dbg_q", q_out, [N2, M]), ("dbg_v1", v1_out, [N2, M]),
            ]
            for (nm, src_t, shp) in dbg_specs:
                dst = nc.declare_dram_parameter(nm, shp, BF16, isOutput=True)
                add("gp", lambda e, dst=dst, src_t=src_t: e.dma_start(
                    out=dst[:, :], in_=src_t[:, :]),
                    deps=[y_dma], extra_waits=[(cc, 6)], inc=16,
                    is_async=True)

        # ---------------- assign sem values --------------------------------
        counters = {"pe": 0, "dv": 0, "ac": 0, "gp": 0, "cc": 0}
        for op in ops:
            if isinstance(op.inc, tuple):
                counters["cc"] += op.inc[1]
                op.sem = cc
                op.val = counters["cc"]
            else:
                counters[op.engine] += op.inc
                op.sem = sem_of[op.engine]
                op.val = counters[op.engine]
        gp_final = counters["gp"]

        def emit_ops(engine_key, e, stage=None):
            for op in ops:
                if op.engine != engine_key:
                    continue
                if engine_key == "pe" and stage is not None and \
                        op.stage != stage:
                    continue
                waits = {}
                for dep in op.deps:
                    key = id(dep.sem)
                    if key not in waits or waits[key][1] < dep.val:
                        waits[key] = (dep.sem, dep.val)
                for (s, v2) in op.extra_waits:
                    key = id(s)
                    if key not in waits or waits[key][1] < v2:
                        waits[key] = (s, v2)
                for (s, v2) in waits.values():
                    e.wait_ge(s, v2)
                ins = op.fn(e)
                if isinstance(op.inc, tuple):
                    ins.then_inc(op.sem, op.inc[1])
                else:
                    ins.then_inc(op.sem, op.inc)

        with nc.Block() as block:

            @block.sync
            def _(sync):
                for (dst, src) in [
                        (w10cat_sb, w10cat_d), (w11r_sb, w11r_d),
                        (w20r_sb, w20r_d), (r0wr_sb, r0wr_d),
                        (r1wr_sb, r1wr_d), (bvec_sb, bvec_d),
                        (ident_sb, ident_d)]:
                    sync.dma_start(out=dst[:, :], in_=src[:, :]).then_inc(
                        s_aux, 16)
                sync.dma_start(
                    out=xT_sb[:, :].rearrange("p (i m) -> p i m", m=M4),
                    in_=xT_d[:, :].rearrange("(i p) m -> p i m", p=128),
                ).then_inc(s_aux, 16)
                sync.dma_start(
                    out=xTs_sb[:, :].rearrange("p (i m) -> p i m", m=M4),
                    in_=xTs_d[:, :].rearrange("(i p) m -> p i m", p=128),
                ).then_inc(s_aux, 16)
                sync.dma_start(
                    out=idx_sb[:, :].rearrange("p (i m) -> p i m", m=DEG),
                    in_=incs_d[:, :].rearrange("(i p) m -> p i m", p=128),
                ).then_inc(s_aux, 16)

                def do_reload(name):
                    cc_src, dest_sb, m_r, cc_val = reload_spec[name]
                    sync.wait_ge(cc, cc_val)
                    sync.dma_start(
                        out=dest_sb[:, :].rearrange("p (i m) -> p i m",
                                                    m=m_r),
                        in_=cc_src[:, :].rearrange("(i p) m -> p i m", p=128),
                    ).then_inc(s_aux, 16)

                # pass A: fill resident s1r
                for i1 in range(C1):
                    if i1 >= NSD:
                        sync.wait_ge(s_dmaN[i1 % NSD], (i1 // NSD) * 16)
                    sync.dma_start(
                        out=s1r[:, i1 * SLAB1:(i1 + 1) * SLAB1],
                        in_=s1s[i1 * 128:(i1 + 1) * 128, :],
                    ).then_inc(s_dmaN[i1 % NSD], 16)
                for name in ("B", "C", "D"):
                    do_reload(name)
                # passes E/F: stream S2 through the ring
                for gs in range(2 * C2):
                    if gs == 8:
                        do_reload("E")
                    if gs == C2 + 8:
                        do_reload("F")
                    if gs >= NB2:
                        sync.wait_ge(pe_free, 256 + gs - NB2 + 1)
                    gd = C1 + gs
                    if gd >= NSD:
                        sync.wait_ge(s_dmaN[gd % NSD], (gd // NSD) * 16)
                    slot = gs % NB2
                    i1 = gs % C2
                    sync.dma_start(
                        out=s2ring[:, slot * SLAB2:(slot + 1) * SLAB2],
                        in_=s2s[i1 * 128:(i1 + 1) * 128, :],
                    ).then_inc(s_dmaN[gd % NSD], 16)

            @block.tensor
            def _(tensor):
                for (name, n_chunks, m, lhsT_sb, streamed, n_mm, stage) in \
                        passes:
                    for i1 in range(n_chunks):
                        if streamed:
                            if name == "A":
                                gd = i1
                            elif name == "E":
                                gd = C1 + i1
                            else:
                                gd = C1 + C2 + i1
                            tensor.wait_ge(s_dmaN[gd % NSD],
                                           (gd // NSD + 1) * 16)
                        if i1 == 0:
                            tensor.wait_ge(s_aux, rhs_ready[name])
                        lhsT = lhsT_sb[:, i1 * m:(i1 + 1) * m]
                        if name in ("E", "F"):
                            slot = (i1 if name == "E" else C2 + i1) % NB2
                            rhs0 = s2ring[:, slot * SLAB2:(slot + 1) * SLAB2]
                        else:
                            rhs0 = s1r[:, i1 * SLAB1:i1 * SLAB1 + 512]
                        ins = tensor.matmul(
                            p0[0:m, 0:512], lhsT, rhs0,
                            start=(i1 == 0), stop=(i1 == n_chunks - 1),
                            skip_group_check=True)
                        if n_mm == 2:
                            ins = tensor.matmul(
                                p1[0:m, 0:512], lhsT,
                                s1r[:, i1 * SLAB1 + 512:(i1 + 1) * SLAB1],
                                start=(i1 == 0), stop=(i1 == n_chunks - 1),
                                skip_group_check=True)
                        ins.then_inc(pe_free, 1)
                    emit_ops("pe", tensor, stage=stage)

            @block.vector
            def _(vector):
                emit_ops("dv", vector)

            @block.scalar
            def _(scalar):
                emit_ops("ac", scalar)

            @block.gpsimd
            def _(gpsimd):
                emit_ops("gp", gpsimd)
                gpsimd.wait_ge(sem_of["gp"], gp_final)

    return nc


def _get_nc():
    if "nc" not in _CACHE:
        _CACHE["nc"] = _build_nc()
    return _CACHE["nc"]


def kernel(x, S1, S2, incid_idx, W10, b10, W11, b11, W20, b20,
           R0W, R0b, R1W, R1b, _results_out=None, **run_kwargs):
    in_maps = _host_prep(x, S1, S2, incid_idx, W10, W11, W20,
                         b10, b11, b20, R0W, R0b, R1W, R1b)
    nc = _get_nc()
    res = run_bass_kernel_spmd(nc, in_maps, CORE_IDS, **run_kwargs)
    if _results_out is not None:
        _results_out.append(res)
    y = np.zeros((B, 2, N2), np.float32)
    for c in range(NC):
        yc = res.results[c]["y"]                      # [8, SLAB2]
        y[:, :, c * SLAB2:(c + 1) * SLAB2] = yc.reshape(B, 2, SLAB2)
    return y


# revision 16
# speedup vs baseline: 2.7732x; 1.0539x over previous
"""LocalGNNCliqueLine Trainium2 kernel, v2: bf16, streamed-S orientation,
S1 slab resident in SBUF.

Reference (B=4, N1=8192, N2=4096, DEG=4, F0=1, F1=F2=16, K=3, R0=16, R1=2):
  h1 = relu(gfilter(x, S1, W10, b10))          # [B,16,N1]
  h2 = relu(gfilter(h1, S1, W11, b11))         # [B,16,N1]
  q  = relu-max-pool(h2, incid_idx)            # [B,16,N2]
  h3 = relu(gfilter(q, S2, W20, b20))          # [B,16,N2]
  y  = (relu(h3^T @ R0W + R0b) @ R1W + R1b)^T  # [B,2,N2]

Sharding: S1/S2 column(=output node)-sharded across 8 cores (slabs cast to
bf16 on the host). Each GSO tap out[m, n2_slab] = z @ S_slab runs with the
small z^T chunk as the PE stationary operand and S chunks as the N=512
moving operand (fp32 PSUM accumulation). The S1 slab is streamed once
during tap 1 into a resident SBUF buffer; taps 2-4 run from SBUF with no
DMA. Slab outputs are PE-transposed back to node-on-partition layout,
AllGathered in bf16, and chained. Feature mixing, pooling (indirect-DMA
gather) and the readout MLP run on slab rows via PE transpose + small
block-diagonal matmuls with ping-pong PSUM banks.
"""

import numpy as np
import ml_dtypes

import concourse.bass as bass
import concourse.mybir as mybir
from concourse.bass_utils import run_bass_kernel_spmd

F32 = mybir.dt.float32
BF16 = mybir.dt.bfloat16
I32 = mybir.dt.int32
NPBF = ml_dtypes.bfloat16

NC = 8
CORE_IDS = list(range(NC))
B, N1, N2, DEG = 4, 8192, 4096, 4
F1 = 16
M4 = B                       # 4 stacked rows for layer-1 taps (F0=1)
M = B * F1                   # 64
SLAB1 = N1 // NC             # 1024
SLAB2 = N2 // NC             # 512
C1 = N1 // 128               # 64
C2 = N2 // 128               # 32
J1 = SLAB1 // 128            # 8
J2 = SLAB2 // 128            # 4
NB2 = 8                      # S2 streaming ring depth

RG = [CORE_IDS]
_CACHE = {}
DEBUG = False


def _host_prep(x, S1, S2, incid_idx, W10, W11, W20, b10, b11, b20,
               R0W, R0b, R1W, R1b):
    f = np.float32
    xT = np.asarray(x, f)[:, 0, :].T                               # [N1, 4]

    w10cat = np.zeros((12, 64), f)                                 # rows (k,b)
    for k in range(3):
        for b in range(4):
            w10cat[k * 4 + b, b * 16:(b + 1) * 16] = np.asarray(W10, f)[k, 0, :]

    def blockdiag3(W):   # [3,16,16] -> [64, 192]; rows (b,f), k-major cols (b,g)
        out = np.zeros((64, 192), f)
        for k in range(3):
            for b in range(4):
                out[b * 16:(b + 1) * 16,
                    k * 64 + b * 16:k * 64 + (b + 1) * 16] = np.asarray(W, f)[k]
        return out

    def blockdiag1(W):   # [16, r] -> [64, 4r]
        W = np.asarray(W, f)
        r = W.shape[1]
        out = np.zeros((64, 4 * r), f)
        for b in range(4):
            out[b * 16:(b + 1) * 16, b * r:(b + 1) * r] = W
        return out

    bvec = np.zeros((64, 8), f)
    bvec[:, 0] = np.tile(np.asarray(b10, f), 4)
    bvec[:, 1] = np.tile(np.asarray(b11, f), 4)
    bvec[:, 2] = np.tile(np.asarray(b20, f), 4)
    bvec[:, 3] = np.tile(np.asarray(R0b, f), 4)
    bvec[:8, 4] = np.tile(np.asarray(R1b, f), 4)

    bf = lambda a: np.ascontiguousarray(np.asarray(a, f).astype(NPBF))  # noqa: E731
    S1 = np.asarray(S1, f)
    S2 = np.asarray(S2, f)
    incid = np.asarray(incid_idx, np.int32)

    common = {
        "xT": bf(xT),
        "w10cat": bf(w10cat), "w11r": bf(blockdiag3(W11)),
        "w20r": bf(blockdiag3(W20)), "r0wr": bf(blockdiag1(R0W)),
        "r1wr": bf(blockdiag1(R1W)), "bvec": np.ascontiguousarray(bvec),
        "ident": bf(np.eye(128, dtype=f)),
    }
    in_maps = []
    for c in range(NC):
        m = dict(common)
        m["s1s"] = bf(S1[:, c * SLAB1:(c + 1) * SLAB1])
        m["s2s"] = bf(S2[:, c * SLAB2:(c + 1) * SLAB2])
        m["xTs"] = bf(xT[c * SLAB1:(c + 1) * SLAB1])
        m["incs"] = np.ascontiguousarray(incid[c * SLAB2:(c + 1) * SLAB2])
        in_maps.append(m)
    return in_maps


class Op:
    __slots__ = ("engine", "fn", "deps", "extra_waits", "inc", "is_async",
                 "stage", "sem", "val")

    def __init__(self, engine, fn, deps=(), extra_waits=(), inc=1,
                 is_async=False, stage=0):
        self.engine = engine
        self.fn = fn
        self.deps = [d for d in deps if d is not None]
        self.extra_waits = list(extra_waits)
        self.inc = inc
        self.is_async = is_async
        self.stage = stage
        self.sem = None
        self.val = None


def _build_nc():
    nc = bass.Bass(num_devices=NC)

    s1s = nc.declare_dram_parameter("s1s", [N1, SLAB1], BF16, isOutput=False)
    s2s = nc.declare_dram_parameter("s2s", [N2, SLAB2], BF16, isOutput=False)
    xT_d = nc.declare_dram_parameter("xT", [N1, M4], BF16, isOutput=False)
    xTs_d = nc.declare_dram_parameter("xTs", [SLAB1, M4], BF16, isOutput=False)
    incs_d = nc.declare_dram_parameter("incs", [SLAB2, DEG], I32, isOutput=False)
    w10cat_d = nc.declare_dram_parameter("w10cat", [12, 64], BF16, isOutput=False)
    w11r_d = nc.declare_dram_parameter("w11r", [64, 192], BF16, isOutput=False)
    w20r_d = nc.declare_dram_parameter("w20r", [64, 192], BF16, isOutput=False)
    r0wr_d = nc.declare_dram_parameter("r0wr", [64, 64], BF16, isOutput=False)
    r1wr_d = nc.declare_dram_parameter("r1wr", [64, 8], BF16, isOutput=False)
    bvec_d = nc.declare_dram_parameter("bvec", [64, 8], F32, isOutput=False)
    ident_d = nc.declare_dram_parameter("ident", [128, 128], BF16, isOutput=False)
    y_d = nc.declare_dram_parameter("y", [8, SLAB2], F32, isOutput=True)

    # rank-block layout: [128, J*m] per rank -> AG concat [128*NC, J*m];
    # reload is then fully contiguous and reload column order == chunk order
    z1_in = nc.dram_tensor("z1_in", [128, J1 * M4], BF16)
    z1_out = nc.dram_tensor("z1_out", [128 * NC, J1 * M4], BF16,
                            addr_space="Shared")
    h_in = nc.dram_tensor("h_in", [128, J1 * M], BF16)
    h_out = nc.dram_tensor("h_out", [128 * NC, J1 * M], BF16,
                           addr_space="Shared")
    u1_in = nc.dram_tensor("u1_in", [128, J1 * M], BF16)
    u1_out = nc.dram_tensor("u1_out", [128 * NC, J1 * M], BF16,
                            addr_space="Shared")
    p_in = nc.dram_tensor("p_in", [SLAB1, M], BF16)        # node-major
    p_out = nc.dram_tensor("p_out", [N1, M], BF16, addr_space="Shared")
    q_in = nc.dram_tensor("q_in", [128, J2 * M], BF16)
    q_out = nc.dram_tensor("q_out", [128 * NC, J2 * M], BF16,
                           addr_space="Shared")
    v1_in = nc.dram_tensor("v1_in", [128, J2 * M], BF16)
    v1_out = nc.dram_tensor("v1_out", [128 * NC, J2 * M], BF16,
                            addr_space="Shared")

    from contextlib import ExitStack
    ctx = ExitStack()
    sb = lambda name, shape, dt=BF16: ctx.enter_context(  # noqa: E731
        nc.sbuf_tensor(name, shape, dt))
    psa = lambda name, dt=F32: ctx.enter_context(  # noqa: E731
        nc.psum_tensor(name, [128, 512], dt))
    mksem = lambda name: ctx.enter_context(nc.semaphore(name))  # noqa: E731

    with ctx:
        s1r = sb("s1r", [128, C1 * SLAB1])            # resident S1 slab bf16
        s2ring = sb("s2ring", [128, NB2 * SLAB2])     # S2 stream ring
        xT_sb = sb("xT_sb", [128, C1 * M4])
        z1T_sb = sb("z1T_sb", [128, C1 * M4])
        xTs_sb = sb("xTs_sb", [128, J1 * M4])
        z1Ts_sb = sb("z1Ts_sb", [128, J1 * M4])
        z2Ts_sb = sb("z2Ts_sb", [128, J1 * M4])
        hT_sb = sb("hT_sb", [128, C1 * M])
        u1T_sb = sb("u1T_sb", [128, C1 * M])
        hTs_sb = sb("hTs_sb", [128, J1 * M])
        u1Ts_sb = sb("u1Ts_sb", [128, J1 * M])
        u2Ts_sb = sb("u2Ts_sb", [128, J1 * M])
        pTs_sb = sb("pTs_sb", [128, J1 * M])
        qT_sb = sb("qT_sb", [128, C2 * M])
        v1T_sb = sb("v1T_sb", [128, C2 * M])
        qTs_sb = sb("qTs_sb", [128, J2 * M])
        v1Ts_sb = sb("v1Ts_sb", [128, J2 * M])
        v2Ts_sb = sb("v2Ts_sb", [128, J2 * M])
        idx_sb = sb("idx_sb", [128, J2 * DEG], I32)
        g_sb = sb("g_sb", [128, (DEG + 2) * M])
        pout_sb = sb("pout_sb", [64, 1024])           # pass output staging
        zc_sb = sb("zc_sb", [128, 2 * 12])            # parity halves
        tr_sb = sb("tr_sb", [64, 2 * 384])
        w_sb = sb("w_sb", [64, 2 * 128])
        a_sb = sb("a_sb", [64, 2 * 128])
        y_sb = sb("y_sb", [8, J2 * 128], F32)
        w10cat_sb = sb("w10cat_sb", [12, 64])
        w11r_sb = sb("w11r_sb", [64, 192])
        w20r_sb = sb("w20r_sb", [64, 192])
        r0wr_sb = sb("r0wr_sb", [64, 64])
        r1wr_sb = sb("r1wr_sb", [64, 8])
        bvec_sb = sb("bvec_sb", [64, 8], F32)
        ident_sb = sb("ident_sb", [128, 128])

        p0 = psa("p0")        # pass accumulator, slab cols 0:512
        p1 = psa("p1")        # pass accumulator, slab cols 512:1024
        ptrb = [psa("ptr0", BF16), psa("ptr1", BF16)]  # epilogue T ping-pong
        ptp = [psa("ptA", BF16), psa("ptB", BF16)]  # combine transposes
        pcp = [psa("pcA"), psa("pcB")]    # combine matmul outputs (f32)

        NSD = 8
        s_dmaN = [mksem(f"s_dma{i}") for i in range(NSD)]
        s_aux = mksem("s_aux")
        pe_free = mksem("pe_free")
        cc = mksem("cc")
        sem_of = {"pe": mksem("pe_c"), "dv": mksem("dv"),
                  "ac": mksem("ac"), "gp": mksem("gp")}

        # ---------------- schedule ----------------------------------------
        # (name, n_chunks, m, lhsT_sb, streamed, n_mm, stage)
        passes = [
            ("A", C1, M4, xT_sb, True, 2, 1),
            ("B", C1, M4, z1T_sb, False, 2, 2),
            ("C", C1, M, hT_sb, False, 2, 3),
            ("D", C1, M, u1T_sb, False, 2, 4),
            ("E", C2, M, qT_sb, True, 1, 5),
            ("F", C2, M, v1T_sb, True, 1, 6),
        ]
        pe_done_val = {}
        v = 0
        for p in passes:
            v += p[1]
            pe_done_val[p[0]] = v

        PRELOAD = 10 * 16
        rhs_ready = {"A": PRELOAD, "B": PRELOAD + 16, "C": PRELOAD + 32,
                     "D": PRELOAD + 48, "E": PRELOAD + 64, "F": PRELOAD + 80}
        reload_spec = {"B": (z1_out, z1T_sb, J1 * M4, 1),
                       "C": (h_out, hT_sb, J1 * M, 2),
                       "D": (u1_out, u1T_sb, J1 * M, 3),
                       "E": (q_out, qT_sb, J2 * M, 5),
                       "F": (v1_out, v1T_sb, J2 * M, 6)}

        ops = []

        def add(engine, fn, deps=(), extra_waits=(), inc=1, is_async=False,
                stage=0):
            op = Op(engine, fn, deps, extra_waits, inc, is_async, stage)
            ops.append(op)
            return op

        AGOP = mybir.AluOpType.bypass
        Relu = mybir.ActivationFunctionType.Relu
        Ident = mybir.ActivationFunctionType.Identity
        Max = mybir.AluOpType.max

        # ---------------- pass epilogues -----------------------------------
        # copy psum -> pout (bf16), PE-transpose slab chunks back to
        # node-on-partition, copy into the slab tensor.
        prev_tp = {}    # j -> last transpose op (WAR for ptr slot j)
        prev_cp = {}    # j -> last slot copy op
        last_tp_of_pass = {}

        def epilogue(name, m, nj, dest_slab, stage):
            def e1fn(e, m=m, nj=nj):
                ins = e.tensor_copy(pout_sb[0:m, 0:512], p0[0:m, 0:512])
                if nj > 4:
                    ins = e.tensor_copy(pout_sb[0:m, 512:1024],
                                        p1[0:m, 0:512])
                return ins
            e1 = add("dv", e1fn, extra_waits=[(pe_free, pe_done_val[name])])
            cps = []
            for j in range(nj):
                pX = ptrb[j % 2]
                # bank guard: PE may not write this ptr bank while the DVE
                # copy two steps back (same bank) might still be reading it
                tp = add("pe", lambda e, j=j, m=m, pX=pX: e.transpose(
                    pX[:, j * 64:j * 64 + m], pout_sb[0:m,
                                                      j * 128:(j + 1) * 128],
                    ident_sb[0:m, 0:m]),
                    deps=[e1, cps[j - 2] if j >= 2 else None],
                    stage=stage)
                cp = add("dv", lambda e, j=j, m=m, pX=pX: e.tensor_copy(
                    dest_slab[:, j * m:(j + 1) * m], pX[:, j * 64:j * 64 + m]),
                    deps=[tp])
                prev_tp[j] = tp
                prev_cp[j] = cp
                cps.append(cp)
            last_tp_of_pass[name] = cps
            return cps

        def stage_ag(name_in, name_out, src_slab, m, cc_deps,
                     node_major=False):
            if node_major:
                d = add("gp", lambda e: e.dma_start(
                    out=name_in[:, :].rearrange("(j p) m -> p j m", p=128),
                    in_=src_slab[:, :].rearrange("p (j m) -> p j m", m=m)),
                    deps=cc_deps, inc=16, is_async=True)
            else:
                d = add("gp", lambda e: e.dma_start(
                    out=name_in[:, :], in_=src_slab[:, :]),
                    deps=cc_deps, inc=16, is_async=True)
            return add("gp", lambda e: e.collective_compute(
                "AllGather", AGOP, replica_groups=RG,
                ins=[name_in[:, :]], outs=[name_out[:, :]]),
                deps=[d], inc=("cc", 1), is_async=True)

        # pass A epilogue + AG z1
        cpsA = epilogue("A", M4, J1, z1Ts_sb, 1)
        stage_ag(z1_in, z1_out, z1Ts_sb, M4, cpsA)

        # pass B epilogue + layer-1 combine + AG h
        cpsB = epilogue("B", M4, J1, z2Ts_sb, 2)
        p_by = {}
        ccs = []
        for jj in range(J1):
            par = jj % 2
            ptX, pcX = ptp[par], pcp[par]
            zc = zc_sb[:, par * 12:par * 12 + 12]
            tr0 = tr_sb[0:12, par * 384:par * 384 + 128]
            wsb = w_sb[:, par * 128:(par + 1) * 128]
            s4 = slice(jj * M4, (jj + 1) * M4)

            def d1fn(e, s4=s4, zc=zc):
                e.tensor_copy(zc[:, 0:4], xTs_sb[:, s4])
                e.tensor_copy(zc[:, 4:8], z1Ts_sb[:, s4])
                return e.tensor_copy(zc[:, 8:12], z2Ts_sb[:, s4])
            d1 = add("dv", d1fn,
                     deps=[cpsA[jj], cpsB[jj], p_by.get(("t1", par))])
            t1 = add("pe", lambda e, zc=zc, ptX=ptX: e.transpose(
                ptX[0:12, 0:128], zc, ident_sb[:, :]),
                deps=[d1, p_by.get(("c3", par))], stage=2)
            c2 = add("dv", lambda e, ptX=ptX, tr0=tr0: e.tensor_copy(
                tr0, ptX[0:12, 0:128]),
                deps=[t1, p_by.get(("m1", par))])
            m1 = add("pe", lambda e, pcX=pcX, tr0=tr0: e.matmul(
                pcX[0:64, 0:128], w10cat_sb[:, :], tr0,
                start=True, stop=True, skip_group_check=True),
                deps=[c2, p_by.get(("a1", par))], stage=2)
            a1 = add("ac", lambda e, pcX=pcX, wsb=wsb: e.activation(
                wsb, pcX[0:64, 0:128], Relu, bias=bvec_sb[:, 0:1]),
                deps=[m1, p_by.get(("t2", par))])
            t2 = add("pe", lambda e, ptX=ptX, wsb=wsb: e.transpose(
                ptX[:, 128:192], wsb, ident_sb[0:64, 0:64]),
                deps=[a1], stage=2)
            c3 = add("dv", lambda e, jj=jj, ptX=ptX: e.tensor_copy(
                hTs_sb[:, jj * M:(jj + 1) * M], ptX[:, 128:192]),
                deps=[t2])
            for k, o in [("t1", t1), ("c3", c3), ("m1", m1), ("a1", a1),
                         ("t2", t2)]:
                p_by[(k, par)] = o
            ccs.append(c3)
        stage_ag(h_in, h_out, hTs_sb, M, ccs)

        # pass C epilogue + AG u1
        cpsC = epilogue("C", M, J1, u1Ts_sb, 3)
        stage_ag(u1_in, u1_out, u1Ts_sb, M, cpsC)

        # pass D epilogue + layer-2 combine + AG p
        cpsD = epilogue("D", M, J1, u2Ts_sb, 4)
        p_by = {}
        ccs = []
        for jj in range(J1):
            par = jj % 2
            ptX, pcX = ptp[par], pcp[par]
            wsb = w_sb[:, par * 128:(par + 1) * 128]
            sM = slice(jj * M, (jj + 1) * M)
            srcs = (hTs_sb, u1Ts_sb, u2Ts_sb)
            src_dep = (None, None, cpsD[jj])
            cks = []
            for k in range(3):
                war = cks[k - 1] if k > 0 else p_by.get(("c4", par))
                tk = add("pe", lambda e, k=k, sM=sM, srcs=srcs, ptX=ptX:
                         e.transpose(ptX[0:64, k * 128:(k + 1) * 128],
                                     srcs[k][:, sM], ident_sb[:, :]),
                         deps=[src_dep[k], war], stage=4)
                ck = add("dv", lambda e, k=k, par=par, ptX=ptX: e.tensor_copy(
                    tr_sb[:, par * 384 + k * 128:par * 384 + (k + 1) * 128],
                    ptX[0:64, k * 128:(k + 1) * 128]),
                    deps=[tk, p_by.get(("m1", par)) if k == 0 else None])
                cks.append(ck)

            def m1fn(e, par=par, pcX=pcX):
                t0 = par * 384
                e.matmul(pcX[0:64, 0:128], w11r_sb[:, 0:64],
                         tr_sb[:, t0:t0 + 128], start=True, stop=False,
                         skip_group_check=True)
                e.matmul(pcX[0:64, 0:128], w11r_sb[:, 64:128],
                         tr_sb[:, t0 + 128:t0 + 256], start=False, stop=False,
                         skip_group_check=True)
                return e.matmul(pcX[0:64, 0:128], w11r_sb[:, 128:192],
                                tr_sb[:, t0 + 256:t0 + 384], start=False,
                                stop=True, skip_group_check=True)
            m1 = add("pe", m1fn, deps=[cks[0], cks[1], cks[2],
                                       p_by.get(("a1", par))], stage=4)
            a1 = add("ac", lambda e, pcX=pcX, wsb=wsb: e.activation(
                wsb, pcX[0:64, 0:128], Relu, bias=bvec_sb[:, 1:2]),
                deps=[m1, p_by.get(("t4", par))])
            t4 = add("pe", lambda e, ptX=ptX, wsb=wsb: e.transpose(
                ptX[:, 384:448], wsb, ident_sb[0:64, 0:64]),
                deps=[a1], stage=4)
            c4 = add("dv", lambda e, jj=jj, ptX=ptX: e.tensor_copy(
                pTs_sb[:, jj * M:(jj + 1) * M], ptX[:, 384:448]),
                deps=[t4])
            for k, o in [("c4", c4), ("m1", m1), ("a1", a1), ("t4", t4)]:
                p_by[(k, par)] = o
            ccs.append(c4)
        ag_p = stage_ag(p_in, p_out, pTs_sb, M, ccs, node_major=True)

        # pooling
        p_mx = None
        mx_ops = []
        for mc in range(J2):
            gg = []
            for d in range(DEG):
                col = mc * DEG + d
                gg.append(add("gp", lambda e, d=d, col=col:
                              e.indirect_dma_start(
                                  out=g_sb[:, d * M:(d + 1) * M],
                                  out_offset=None,
                                  in_=p_out[:, :],
                                  in_offset=bass.IndirectOffsetOnAxis(
                                      ap=idx_sb[:, col:col + 1], axis=0)),
                              deps=[ag_p, p_mx], inc=16, is_async=True))

            def mxfn(e, mc=mc):
                e.tensor_tensor(g_sb[:, 4 * M:5 * M], g_sb[:, 0:M],
                                g_sb[:, M:2 * M], op=Max)
                e.tensor_tensor(g_sb[:, 5 * M:6 * M], g_sb[:, 2 * M:3 * M],
                                g_sb[:, 3 * M:4 * M], op=Max)
                e.drain()
                e.tensor_tensor(g_sb[:, 4 * M:5 * M], g_sb[:, 4 * M:5 * M],
                                g_sb[:, 5 * M:6 * M], op=Max)
                e.drain()
                return e.tensor_scalar_max(
                    qTs_sb[:, mc * M:(mc + 1) * M], g_sb[:, 4 * M:5 * M], 0.0)
            p_mx = add("dv", mxfn, deps=gg)
            mx_ops.append(p_mx)
        stage_ag(q_in, q_out, qTs_sb, M, [mx_ops[-1]])

        # pass E epilogue + AG v1
        cpsE = epilogue("E", M, J2, v1Ts_sb, 5)
        stage_ag(v1_in, v1_out, v1Ts_sb, M, cpsE)

        # pass F epilogue + stage-2 combine + readout
        cpsF = epilogue("F", M, J2, v2Ts_sb, 6)
        p_by = {}
        a3_ops = []
        for jj in range(J2):
            par = jj % 2
            ptX, pcX = ptp[par], pcp[par]
            wsb = w_sb[:, par * 128:(par + 1) * 128]
            asb = a_sb[:, par * 128:(par + 1) * 128]
            sM = slice(jj * M, (jj + 1) * M)
            srcs = (qTs_sb, v1Ts_sb, v2Ts_sb)
            src_dep = (mx_ops[jj], cpsE[jj], cpsF[jj])
            cks = []
            for k in range(3):
                war = cks[k - 1] if k > 0 else p_by.get(("c4", par))
                tk = add("pe", lambda e, k=k, sM=sM, srcs=srcs, ptX=ptX:
                         e.transpose(ptX[0:64, k * 128:(k + 1) * 128],
                                     srcs[k][:, sM], ident_sb[:, :]),
                         deps=[src_dep[k], war], stage=6)
                ck = add("dv", lambda e, k=k, par=par, ptX=ptX: e.tensor_copy(
                    tr_sb[:, par * 384 + k * 128:par * 384 + (k + 1) * 128],
                    ptX[0:64, k * 128:(k + 1) * 128]),
                    deps=[tk, p_by.get(("m1", par)) if k == 0 else None])
                cks.append(ck)
            # no back-transpose in stage 6; c4 key tracks last ptX reader
            p_by[("c4", par)] = cks[2]

            def m1fn(e, par=par, pcX=pcX):
                t0 = par * 384
                e.matmul(pcX[0:64, 0:128], w20r_sb[:, 0:64],
                         tr_sb[:, t0:t0 + 128], start=True, stop=False,
                         skip_group_check=True)
                e.matmul(pcX[0:64, 0:128], w20r_sb[:, 64:128],
                         tr_sb[:, t0 + 128:t0 + 256], start=False, stop=False,
                         skip_group_check=True)
                return e.matmul(pcX[0:64, 0:128], w20r_sb[:, 128:192],
                                tr_sb[:, t0 + 256:t0 + 384], start=False,
                                stop=True, skip_group_check=True)
            m1 = add("pe", m1fn, deps=[cks[0], cks[1], cks[2],
                                       p_by.get(("a3", par))], stage=6)
            a1 = add("ac", lambda e, pcX=pcX, wsb=wsb: e.activation(
                wsb, pcX[0:64, 0:128], Relu, bias=bvec_sb[:, 2:3]),
                deps=[m1, p_by.get(("m2", par))])
            m2 = add("pe", lambda e, pcX=pcX, wsb=wsb: e.matmul(
                pcX[0:64, 128:256], r0wr_sb[:, :], wsb,
                start=True, stop=True, skip_group_check=True),
                deps=[a1, p_by.get(("a2", par))], stage=6)
            a2 = add("ac", lambda e, pcX=pcX, asb=asb: e.activation(
                asb, pcX[0:64, 128:256], Relu, bias=bvec_sb[:, 3:4]),
                deps=[m2, p_by.get(("m3", par))])
            m3 = add("pe", lambda e, pcX=pcX, asb=asb: e.matmul(
                pcX[0:8, 256:384], r1wr_sb[:, :], asb,
                start=True, stop=True, skip_group_check=True),
                deps=[a2, p_by.get(("a3", par))], stage=6)
            a3 = add("ac", lambda e, jj=jj, pcX=pcX: e.activation(
                y_sb[:, jj * 128:(jj + 1) * 128], pcX[0:8, 256:384], Ident,
                bias=bvec_sb[0:8, 4:5]), deps=[m3])
            for k, o in [("m1", m1), ("m2", m2), ("m3", m3), ("a2", a2),
                         ("a3", a3)]:
                p_by[(k, par)] = o
            a3_ops.append(a3)
        y_dma = add("gp", lambda e: e.dma_start(out=y_d[:, :], in_=y_sb[:, :]),
                    deps=list(a3_ops), inc=16, is_async=True)

        if DEBUG:
            dbg_specs = [
                ("dbg_z1", z1_out, [N1, M4]), ("dbg_h", h_out, [N1, M]),
                ("dbg_u1", u1_out, [N1, M]), ("dbg_p", p_out, [N1, M]),
                ("dbg_q", q_out, [N2, M]), ("dbg_v1", v1_out, [N2, M]),
            ]
            for (nm, src_t, shp) in dbg_specs:
                dst = nc.declare_dram_parameter(nm, shp, BF16, isOutput=True)
                add("gp", lambda e, dst=dst, src_t=src_t: e.dma_start(
                    out=dst[:, :], in_=src_t[:, :]),
                    deps=[y_dma], extra_waits=[(cc, 6)], inc=16,
                    is_async=True)

        # ---------------- assign sem values --------------------------------
        counters = {"pe": 0, "dv": 0, "ac": 0, "gp": 0, "cc": 0}
        for op in ops:
            if isinstance(op.inc, tuple):
                counters["cc"] += op.inc[1]
                op.sem = cc
                op.val = counters["cc"]
            else:
                counters[op.engine] += op.inc
                op.sem = sem_of[op.engine]
                op.val = counters[op.engine]
        gp_final = counters["gp"]

        def emit_ops(engine_key, e, stage=None):
            for op in ops:
                if op.engine != engine_key:
                    continue
                if engine_key == "pe" and stage is not None and \
                        op.stage != stage:
                    continue
                waits = {}
                for dep in op.deps:
                    key = id(dep.sem)
                    if key not in waits or waits[key][1] < dep.val:
                        waits[key] = (dep.sem, dep.val)
                for (s, v2) in op.extra_waits:
                    key = id(s)
                    if key not in waits or waits[key][1] < v2:
                        waits[key] = (s, v2)
                for (s, v2) in waits.values():
                    e.wait_ge(s, v2)
                ins = op.fn(e)
                if isinstance(op.inc, tuple):
                    ins.then_inc(op.sem, op.inc[1])
                else:
                    ins.then_inc(op.sem, op.inc)

        with nc.Block() as block:

            @block.sync
            def _(sync):
                for (dst, src) in [
                        (w10cat_sb, w10cat_d), (w11r_sb, w11r_d),
                        (w20r_sb, w20r_d), (r0wr_sb, r0wr_d),
                        (r1wr_sb, r1wr_d), (bvec_sb, bvec_d),
                        (ident_sb, ident_d)]:
                    sync.dma_start(out=dst[:, :], in_=src[:, :]).then_inc(
                        s_aux, 16)
                sync.dma_start(
                    out=xT_sb[:, :].rearrange("p (i m) -> p i m", m=M4),
                    in_=xT_d[:, :].rearrange("(i p) m -> p i m", p=128),
                ).then_inc(s_aux, 16)
                sync.dma_start(
                    out=xTs_sb[:, :].rearrange("p (i m) -> p i m", m=M4),
                    in_=xTs_d[:, :].rearrange("(i p) m -> p i m", p=128),
                ).then_inc(s_aux, 16)
                sync.dma_start(
                    out=idx_sb[:, :].rearrange("p (i m) -> p i m", m=DEG),
                    in_=incs_d[:, :].rearrange("(i p) m -> p i m", p=128),
                ).then_inc(s_aux, 16)

                def do_reload(name):
                    cc_src, dest_sb, w_r, cc_val = reload_spec[name]
                    sync.wait_ge(cc, cc_val)
                    sync.dma_start(
                        out=dest_sb[:, :].rearrange("p (r w) -> p r w",
                                                    w=w_r),
                        in_=cc_src[:, :].rearrange("(r p) w -> p r w", p=128),
                    ).then_inc(s_aux, 16)

                # pass A: fill resident s1r
                for i1 in range(C1):
                    if i1 >= NSD:
                        sync.wait_ge(s_dmaN[i1 % NSD], (i1 // NSD) * 16)
                    sync.dma_start(
                        out=s1r[:, i1 * SLAB1:(i1 + 1) * SLAB1],
                        in_=s1s[i1 * 128:(i1 + 1) * 128, :],
                    ).then_inc(s_dmaN[i1 % NSD], 16)
                for name in ("B", "C", "D"):
                    do_reload(name)
                # passes E/F: stream S2 through the ring
                for gs in range(2 * C2):
                    if gs == 8:
                        do_reload("E")
                    if gs == C2 + 8:
                        do_reload("F")
                    if gs >= NB2:
                        sync.wait_ge(pe_free, 256 + gs - NB2 + 1)
                    gd = C1 + gs
                    if gd >= NSD:
                        sync.wait_ge(s_dmaN[gd % NSD], (gd // NSD) * 16)
                    slot = gs % NB2
                    i1 = gs % C2
                    sync.dma_start(
                        out=s2ring[:, slot * SLAB2:(slot + 1) * SLAB2],
                        in_=s2s[i1 * 128:(i1 + 1) * 128, :],
                    ).then_inc(s_dmaN[gd % NSD], 16)

            @block.tensor
            def _(tensor):
                for (name, n_chunks, m, lhsT_sb, streamed, n_mm, stage) in \
                        passes:
                    for i1 in range(n_chunks):
                        if streamed:
                            if name == "A":
                                gd = i1
                            elif name == "E":
                                gd = C1 + i1
                            else:
                                gd = C1 + C2 + i1
                            tensor.wait_ge(s_dmaN[gd % NSD],
                                           (gd // NSD + 1) * 16)
                        if i1 == 0:
                            tensor.wait_ge(s_aux, rhs_ready[name])
                        lhsT = lhsT_sb[:, i1 * m:(i1 + 1) * m]
                        if name in ("E", "F"):
                            slot = (i1 if name == "E" else C2 + i1) % NB2
                            rhs0 = s2ring[:, slot * SLAB2:(slot + 1) * SLAB2]
                        else:
                            rhs0 = s1r[:, i1 * SLAB1:i1 * SLAB1 + 512]
                        ins = tensor.matmul(
                            p0[0:m, 0:512], lhsT, rhs0,
                            start=(i1 == 0), stop=(i1 == n_chunks - 1),
                            skip_group_check=True)
                        if n_mm == 2:
                            ins = tensor.matmul(
                                p1[0:m, 0:512], lhsT,
                                s1r[:, i1 * SLAB1 + 512:(i1 + 1) * SLAB1],
                                start=(i1 == 0), stop=(i1 == n_chunks - 1),
                                skip_group_check=True)
                        ins.then_inc(pe_free, 1)
                    emit_ops("pe", tensor, stage=stage)

            @block.vector
            def _(vector):
                emit_ops("dv", vector)

            @block.scalar
            def _(scalar):
                emit_ops("ac", scalar)

            @block.gpsimd
            def _(gpsimd):
                emit_ops("gp", gpsimd)
                gpsimd.wait_ge(sem_of["gp"], gp_final)

    return nc


def _get_nc():
    if "nc" not in _CACHE:
        _CACHE["nc"] = _build_nc()
    return _CACHE["nc"]


def kernel(x, S1, S2, incid_idx, W10, b10, W11, b11, W20, b20,
           R0W, R0b, R1W, R1b, _results_out=None, **run_kwargs):
    in_maps = _host_prep(x, S1, S2, incid_idx, W10, W11, W20,
                         b10, b11, b20, R0W, R0b, R1W, R1b)
    nc = _get_nc()
    res = run_bass_kernel_spmd(nc, in_maps, CORE_IDS, **run_kwargs)
    if _results_out is not None:
        _results_out.append(res)
    y = np.zeros((B, 2, N2), np.float32)
    for c in range(NC):
        yc = res.results[c]["y"]                      # [8, SLAB2]
        y[:, :, c * SLAB2:(c + 1) * SLAB2] = yc.reshape(B, 2, SLAB2)
    return y
